# revision 17
# baseline (speedup 1.0000x reference)
"""AttnBlock (GroupNorm -> QKV 1x1 -> full NxN attention -> out-proj + residual)
on 8 Trainium2 NeuronCores, data-parallel over batch (1 batch element/core).

Shapes (hardcoded): x [8, 256, 64, 64] f32, c=256, n=h*w=4096.

Per-core scheme (all on-chip, no transposes):
  - x, hn, q, k live in [c, n] layout: SBUF [128 part, 2 c-chunks, 4096].
  - scores computed transposed: sT[j, i] = sum_c k[c,j] q[c,i] (lhsT=k, rhs=q),
    softmax without max-subtraction (scores are ~N(0,1) after the 1/16 scale
    folded into wq), exp on ScalarE PSUM->SBUF (2 j-tiles per ACT op).
  - v is produced pre-transposed with the output projection folded in:
    v'T[j, co] = sum_ci hn[ci, j] * (wo@wv).T[ci, co]  (lhsT=hn, rhs=wovT).
  - out[co, i] = sum_j v'T[j, co] * exp(sT)[j, i] accumulates in PSUM over j;
    softmax denominators via partial sums of exp tiles (split between VectorE
    and GpSimd), folded to one [128,512] tile, then one ones[128,128] f32r
    matmul that both reduces over partitions and broadcasts to 128 partitions;
    division by the denominator + residual add fused into PSUM eviction.
  - biases: bq, bk applied at PSUM eviction (per-partition); bv/bo folded
    host-side into bo' = bo + wo@bv (softmax rows sum to 1).
"""

import sys

if "/opt/trn_rl_repo" not in sys.path:
    sys.path.insert(0, "/opt/trn_rl_repo")

import numpy as np

P = 128
C = 256
CC = C // P          # 2 channel chunks
H = W = 64
N = H * W            # 4096
NJT = N // P         # 32 j-tiles of 128
IB = 512             # i-block (psum bank width)
NIB = N // IB        # 8 i-blocks
JG = 2               # j-tiles per exp group
NGRP = NJT // JG     # exp groups per i-block
XCH = 8              # x DMA / GN chunks
G = 8                # groups
EPS = 1e-6
NG = (C // G) * N    # elements per group

_CACHE = {}


def _build():
    import concourse.tile as tile
    from concourse import bacc, mybir
    from concourse.bass_interp import get_hw_module

    f32 = mybir.dt.float32
    f32r = mybir.dt.float32r
    AF = mybir.ActivationFunctionType
    AX = mybir.AxisListType
    OP = mybir.AluOpType

    nc = bacc.Bacc("TRN2", target_bir_lowering=False, debug=False,
                   enable_asserts=False, num_devices=1)

    x_d = nc.dram_tensor("x", (C, N), f32, kind="ExternalInput").ap()
    ws_d = nc.dram_tensor("wstack", (3, C, C), f32, kind="ExternalInput").ap()
    bs_d = nc.dram_tensor("bstack", (5, C), f32, kind="ExternalInput").ap()
    g_d = nc.dram_tensor("Gm", (CC, P, G), f32, kind="ExternalInput").ap()
    gt_d = nc.dram_tensor("GmT", (CC, G, P), f32, kind="ExternalInput").ap()
    out_d = nc.dram_tensor("out", (C, N), f32, kind="ExternalOutput").ap()

    x_r = x_d.rearrange("(cc p) n -> p cc n", p=P)
    out_r = out_d.rearrange("(cc p) n -> p cc n", p=P)

    # index of the one ACT table set covering every func we use
    # (ln, exp, square, copy, identity)
    from concourse.hw_specs import get_activation_tables
    act_sets = list(get_activation_tables(nc.m.arch))
    LNEXP_SET = act_sets.index("natural_log_exp_and_others")

    with tile.TileContext(nc) as tc:
        with (
            tc.tile_pool(name="const", bufs=1) as const,
            tc.tile_pool(name="data", bufs=1) as data,
            tc.tile_pool(name="ps", bufs=1, space="PSUM") as ps,
        ):
            # preload it once at t=0 so no mid-kernel table switches occur
            _ld = mybir.InstLoadActFuncSet(
                name=nc.get_next_instruction_name(), ins=[], outs=[],
                act_func_set_id=LNEXP_SET)
            nc.scalar.add_instruction(_ld)
            # ---- load x in chunks first (DMA-critical), GN stats overlap ----
            NC_ = N // XCH
            xt = data.tile([P, CC, N], f32, tag="x")
            q_t = data.tile([P, CC, N], f32r, tag="q")
            k_t = data.tile([P, CC, N], f32r, tag="k")
            vp_t = data.tile([P, NJT, C], f32r, tag="vp")

            with tc.tile_pool(name="hnp", bufs=1) as hnp:
                hn = hnp.tile([P, CC, N], f32r, tag="hn")

                # per-chunk bn_stats records: two (count, mean, count*var)
                # triples per 512-wide half
                st6 = hnp.tile([P, CC, XCH, 6], f32, tag="st6")
                for xc in range(XCH):
                    nsl = slice(xc * NC_, (xc + 1) * NC_)
                    nc.sync.dma_start(xt[:, :, nsl], x_r[:, :, nsl])
                    for cc in range(CC):
                        nc.vector.bn_stats(st6[:, cc, xc], xt[:, cc, nsl])

                # ---- constants (after x on the DMA queue) ----
                w_l = hnp.tile([P, 3, CC, C], f32, tag="wl")
                nc.sync.dma_start(w_l[:], ws_d.rearrange("w (cc p) o -> p w cc o", p=P))
                b_t = const.tile([P, 5, CC], f32, tag="bt")
                nc.sync.dma_start(b_t[:], bs_d.rearrange("v (cc p) -> p v cc", p=P))
                g_t = const.tile([P, CC, G], f32, tag="G")
                nc.sync.dma_start(g_t[:], g_d.rearrange("cc p g -> p cc g"))
                gt_t = const.tile([G, CC, P], f32, tag="GT")
                nc.sync.dma_start(gt_t[:], gt_d.rearrange("cc g p -> g cc p"))

                # rounding copies on ScalarE/GpSimd: VectorE owns the
                # GN-stats critical path at startup
                w_r = const.tile([P, 3, CC, C], f32r, tag="wr")
                wq_t, wk_t, wov_t = w_r[:, 0], w_r[:, 1], w_r[:, 2]
                nc.scalar.activation(w_r[:, 1], w_l[:, 1], AF.Copy)   # k first
                nc.scalar.activation(w_r[:, 0], w_l[:, 0], AF.Copy)
                nc.scalar.activation(w_r[:, 2], w_l[:, 2], AF.Copy)
                bq_t, bk_t, bo_t, gns_t, gnb_t = (b_t[:, v] for v in range(5))

                ones_l = hnp.tile([P, P], f32, tag="onesl")
                nc.gpsimd.memset(ones_l[:], 1.0)
                ones_t = const.tile([P, P], f32r, tag="ones")
                nc.gpsimd.tensor_copy(ones_t[:], ones_l[:])
                eps_t = const.tile([G, 1], f32, tag="eps")
                nc.gpsimd.memset(eps_t[:], EPS)

                # per-channel (mean, E[x^2]) from aggregated bn records
                mv = const.tile([P, CC, 2], f32, tag="mv")
                stc = const.tile([P, CC, 2], f32, tag="stc")
                for cc in range(CC):
                    nc.vector.bn_aggr(mv[:, cc], st6[:, cc])
                    nc.vector.tensor_copy(stc[:, cc, 0:1], mv[:, cc, 0:1])
                    nc.vector.tensor_mul(stc[:, cc, 1:2], mv[:, cc, 0:1],
                                         mv[:, cc, 0:1])
                    nc.vector.tensor_add(stc[:, cc, 1:2], stc[:, cc, 1:2],
                                         mv[:, cc, 1:2])

                # group-reduce per-channel (mean, E[x^2]) straight in PSUM
                gps = ps.tile([G, 2], f32, tag="ob", bufs=2, name="gps")
                for cc in range(CC):
                    nc.tensor.matmul(gps[:], g_t[:, cc], stc[:, cc],
                                     start=(cc == 0), stop=(cc == CC - 1))

                # grp cols: 0=mean 1=rstd 2=ex2 3=mean^2 4=var 5=sqrt(var+eps)
                CPG = C // G
                grp = const.tile([G, 6], f32, tag="grp")
                nc.vector.tensor_scalar_mul(grp[:, 0:1], gps[:, 0:1], 1.0 / CPG)
                nc.vector.tensor_scalar_mul(grp[:, 2:3], gps[:, 1:2], 1.0 / CPG)
                nc.vector.tensor_mul(grp[:, 3:4], grp[:, 0:1], grp[:, 0:1])
                nc.vector.tensor_sub(grp[:, 4:5], grp[:, 2:3], grp[:, 3:4])
                # rstd = exp(-0.5*ln(var+eps)): keeps every ACT func in the
                # natural_log_exp set -> a single table load for the kernel
                nc.scalar.activation(grp[:, 5:6], grp[:, 4:5], AF.Ln, bias=eps_t[:])
                nc.scalar.activation(grp[:, 1:2], grp[:, 5:6], AF.Exp, scale=-0.5)

                # broadcast group (mean, rstd) to channels; A/B affine coeffs
                ab = const.tile([P, CC, 2], f32, tag="ab")  # 0=A 1=B
                for cc in range(CC):
                    chps = ps.tile([P, 2], f32, tag="ob", bufs=2, name="chps")
                    nc.tensor.matmul(chps[:], gt_t[:, cc], grp[:, 0:2],
                                     start=True, stop=True)
                    # A = rstd * gn_scale
                    nc.vector.tensor_mul(ab[:, cc, 0:1], chps[:, 1:2],
                                         gns_t[:, cc:cc + 1])
                    # B = gn_bias - mean * A
                    nc.vector.tensor_mul(ab[:, cc, 1:2], chps[:, 0:1], ab[:, cc, 0:1])
                    nc.vector.tensor_sub(ab[:, cc, 1:2], gnb_t[:, cc:cc + 1],
                                         ab[:, cc, 1:2])

                # hn = A*x + B, split across ScalarE and VectorE
                for xc in range(XCH):
                    for cc in range(CC):
                        nsl = slice(xc * NC_, (xc + 1) * NC_)
                        if xc % 2 == 0:
                            nc.scalar.activation(hn[:, cc, nsl], xt[:, cc, nsl],
                                                 AF.Identity,
                                                 bias=ab[:, cc, 1:2],
                                                 scale=ab[:, cc, 0:1])
                        else:
                            nc.vector.tensor_scalar(hn[:, cc, nsl], xt[:, cc, nsl],
                                                    ab[:, cc, 0:1], ab[:, cc, 1:2],
                                                    OP.mult, OP.add)

                # xt += bo' (residual base) on GpSimd; only needs raw x and
                # runs during the projection phase when Pool is idle
                for cc in range(CC):
                    nc.gpsimd.tensor_scalar_add(xt[:, cc], xt[:, cc],
                                                bo_t[:, cc:cc + 1])

                # ---- k projection (evictions on ScalarE) ----
                for oc in range(CC):
                    for ib in range(NIB):
                        pp = ps.tile([P, IB], f32, tag="ob", bufs=2, name="pp")
                        for ci in range(CC):
                            nc.tensor.matmul(
                                pp[:],
                                wk_t[:, ci, oc * P:(oc + 1) * P],
                                hn[:, ci, ib * IB:(ib + 1) * IB],
                                start=(ci == 0), stop=(ci == CC - 1))
                        nc.scalar.activation(k_t[:, oc, ib * IB:(ib + 1) * IB],
                                             pp[:], AF.Identity,
                                             bias=bk_t[:, oc:oc + 1])

                # ---- v'T[j, co] = sum_ci hn[ci, j] wovT[ci, co] (no bias) ----
                for jt in range(NJT):
                    pv = ps.tile([P, C], f32, tag="ob", bufs=2, name="pv")
                    for ci in range(CC):
                        nc.tensor.matmul(
                            pv[:],
                            hn[:, ci, jt * P:(jt + 1) * P],
                            wov_t[:, ci, :],
                            start=(ci == 0), stop=(ci == CC - 1))
                    nc.vector.tensor_copy(vp_t[:, jt], pv[:])

                # ---- q projection, i-blocks outermost (attention starts on
                # ib0 as soon as its q slice lands); evictions on VectorE ----
                for ib in range(NIB):
                    for oc in range(CC):
                        pq = ps.tile([P, IB], f32, tag="ob", bufs=2, name="pq")
                        for ci in range(CC):
                            nc.tensor.matmul(
                                pq[:],
                                wq_t[:, ci, oc * P:(oc + 1) * P],
                                hn[:, ci, ib * IB:(ib + 1) * IB],
                                start=(ci == 0), stop=(ci == CC - 1))
                        nc.vector.tensor_scalar_add(
                            q_t[:, oc, ib * IB:(ib + 1) * IB], pq[:],
                            bq_t[:, oc:oc + 1])

            with tc.tile_pool(name="work", bufs=1) as work:
                for ib in range(NIB):
                    isl = slice(ib * IB, (ib + 1) * IB)
                    ob = []
                    for co in range(CC):
                        obt = ps.tile([P, IB], f32, tag="ob", bufs=2,
                                      name=f"ob_{ib}_{co}")
                        ob.append(obt)
                    esa = work.tile([P, JG, IB], f32r, tag="esum", bufs=4, name="esa")
                    esb = work.tile([P, JG, IB], f32r, tag="esum", bufs=4, name="esb")

                    for g in range(NGRP):
                        ssg = ps.tile([P, JG, IB], f32, tag="score", bufs=3, name="ssg")
                        for t in range(JG):
                            jt = g * JG + t
                            for ci in range(CC):
                                nc.tensor.matmul(
                                    ssg[:, t],
                                    k_t[:, ci, jt * P:(jt + 1) * P],
                                    q_t[:, ci, isl],
                                    start=(ci == 0), stop=(ci == CC - 1))
                        et = work.tile([P, JG, IB], f32r, tag="exp", bufs=4, name="et")
                        nc.scalar.activation(et[:], ssg[:], AF.Exp)
                        for t in range(JG):
                            jt = g * JG + t
                            for co in range(CC):
                                nc.tensor.matmul(
                                    ob[co][:],
                                    vp_t[:, jt, co * P:(co + 1) * P],
                                    et[:, t],
                                    start=(jt == 0), stop=(jt == NJT - 1))
                        last_direct = (ib == NIB - 1 and g == NGRP - 1)
                        if last_direct:
                            et_last = et
                        elif g == 0:
                            nc.gpsimd.tensor_copy(esb[:], et[:].bitcast(f32))
                        elif g < NGRP // 2:
                            nc.gpsimd.tensor_add(esb[:], esb[:].bitcast(f32),
                                                 et[:].bitcast(f32))
                        elif g == NGRP // 2:
                            nc.vector.tensor_copy(esa[:], et[:].bitcast(f32))
                        else:
                            nc.vector.tensor_add(esa[:], esa[:].bitcast(f32),
                                                 et[:].bitcast(f32))

                    # free the ob psum slots right away via ScalarE copies
                    obs = []
                    for co in range(CC):
                        ot = work.tile([P, IB], f32, tag="obs", bufs=4,
                                       name=f"obs_{ib}_{co}")
                        nc.scalar.activation(ot[:], ob[co][:], AF.Copy)
                        obs.append(ot)

                    # denominators: partials reduced over partitions (and
                    # broadcast to 128 partitions) by accumulating matmuls
                    smt = ps.tile([P, JG, IB], f32, tag="score", bufs=3, name="smt")
                    sm = smt[:, 0]
                    parts = [esb, esa] + ([et_last] if ib == NIB - 1 else [])
                    for z, es in enumerate(parts):
                        nc.tensor.matmul(sm, ones_t[:], es[:, 0],
                                         start=(z == 0), stop=False)
                        nc.tensor.matmul(sm, ones_t[:], es[:, 1],
                                         start=False, stop=(z == len(parts) - 1))
                    rec = work.tile([P, IB], f32, tag="rec", bufs=2, name="rec")
                    nc.vector.reciprocal(rec[:], sm)

                    for co in range(CC):
                        on_t = work.tile([P, IB], f32, tag="on", bufs=3, name="on_t")
                        nc.vector.tensor_mul(on_t[:], obs[co][:], rec[:])
                        fin = work.tile([P, IB], f32, tag="fin", bufs=3, name="fin")
                        if ib == NIB - 1 and co == 0:
                            nc.gpsimd.tensor_add(fin[:], on_t[:], xt[:, co, isl])
                        else:
                            nc.vector.tensor_add(fin[:], on_t[:], xt[:, co, isl])
                        nc.sync.dma_start(out_r[:, co, isl], fin[:])

    nc.compile()
    nc.m = get_hw_module(nc.m)
    return nc


def _get_nc():
    if "nc" not in _CACHE:
        _CACHE["nc"] = _build()
    return _CACHE["nc"]


def _prep_inputs(x, gn_scale, gn_bias, wq, bq, wk, bk, wv, bv, wo, bo):
    f = np.float32
    x = np.asarray(x, f)
    b = x.shape[0]
    scale = 1.0 / np.sqrt(np.float64(C))
    wqT = (np.asarray(wq, np.float64) * scale).T
    bq2 = (np.asarray(bq, np.float64) * scale).astype(f)
    wkT = np.asarray(wk, np.float64).T
    wovT = (np.asarray(wo, np.float64) @ np.asarray(wv, np.float64)).T
    bo2 = (np.asarray(bo, np.float64)
           + np.asarray(wo, np.float64) @ np.asarray(bv, np.float64)).astype(f)
    wstack = np.ascontiguousarray(
        np.stack([wqT, wkT, wovT]).astype(f))
    bstack = np.ascontiguousarray(np.stack(
        [bq2, np.asarray(bk, f), bo2, np.asarray(gn_scale, f),
         np.asarray(gn_bias, f)]))

    gm = np.zeros((CC, P, G), f)
    for cc in range(CC):
        for p in range(P):
            gm[cc, p, (cc * P + p) // (C // G)] = 1.0
    gmT = np.ascontiguousarray(np.transpose(gm, (0, 2, 1)))

    shared = {"wstack": wstack, "bstack": bstack, "Gm": gm, "GmT": gmT}
    in_maps = []
    for i in range(b):
        m = dict(shared)
        m["x"] = np.ascontiguousarray(x[i].reshape(C, N))
        in_maps.append(m)
    return in_maps


def _run(in_maps, trace=False, trace_cores=None):
    from concourse import bass_utils
    nc = _get_nc()
    return bass_utils.run_bass_kernel_spmd(
        nc, in_maps, core_ids=list(range(len(in_maps))),
        trace=trace, trace_cores=trace_cores)


def kernel(x, gn_scale, gn_bias, wq, bq, wk, bk, wv, bv, wo, bo):
    in_maps = _prep_inputs(x, gn_scale, gn_bias, wq, bq, wk, bk, wv, bv, wo, bo)
    res = _run(in_maps)
    b = np.asarray(x).shape[0]
    out = np.stack([res.results[i]["out"].reshape(C, H, W) for i in range(b)])
    return out.astype(np.float32)


# revision 29
# speedup vs baseline: 1.2272x; 1.2272x over previous
"""AttnBlock (GroupNorm -> QKV 1x1 -> full NxN attention -> out-proj + residual)
on 8 Trainium2 NeuronCores, data-parallel over batch (1 batch element/core).

Shapes (hardcoded): x [8, 256, 64, 64] f32, c=256, n=h*w=4096.

Per-core scheme (all on-chip, no transposes):
  - x, hn, q, k live in [c, n] layout: SBUF [128 part, 2 c-chunks, 4096].
  - scores computed transposed: sT[j, i] = sum_c k[c,j] q[c,i] (lhsT=k, rhs=q),
    softmax without max-subtraction (scores are ~N(0,1) after the 1/16 scale
    folded into wq), exp on ScalarE PSUM->SBUF (2 j-tiles per ACT op).
  - v is produced pre-transposed with the output projection folded in:
    v'T[j, co] = sum_ci hn[ci, j] * (wo@wv).T[ci, co]  (lhsT=hn, rhs=wovT).
  - out[co, i] = sum_j v'T[j, co] * exp(sT)[j, i] accumulates in PSUM over j;
    softmax denominators via partial sums of exp tiles (split between VectorE
    and GpSimd), folded to one [128,512] tile, then one ones[128,128] f32r
    matmul that both reduces over partitions and broadcasts to 128 partitions;
    division by the denominator + residual add fused into PSUM eviction.
  - biases: bq, bk applied at PSUM eviction (per-partition); bv/bo folded
    host-side into bo' = bo + wo@bv (softmax rows sum to 1).
"""

import sys

if "/opt/trn_rl_repo" not in sys.path:
    sys.path.insert(0, "/opt/trn_rl_repo")

import numpy as np

P = 128
C = 256
CC = C // P          # 2 channel chunks
H = W = 64
N = H * W            # 4096
NJT = N // P         # 32 j-tiles of 128
IB = 512             # i-block (psum bank width)
NIB = N // IB        # 8 i-blocks
JG = 2               # j-tiles per exp group
NGRP = NJT // JG     # exp groups per i-block
XCH = 8              # x DMA / GN chunks
G = 8                # groups
EPS = 1e-6
NG = (C // G) * N    # elements per group

_CACHE = {}


def _build():
    import concourse.tile as tile
    from concourse import bacc, mybir
    from concourse.bass_interp import get_hw_module

    f32 = mybir.dt.float32
    f32r = mybir.dt.float32r
    AF = mybir.ActivationFunctionType
    AX = mybir.AxisListType
    OP = mybir.AluOpType

    nc = bacc.Bacc("TRN2", target_bir_lowering=False, debug=False,
                   enable_asserts=False, num_devices=1)

    x_d = nc.dram_tensor("x", (C, N), f32, kind="ExternalInput").ap()
    ws_d = nc.dram_tensor("wstack", (3, C, C), f32, kind="ExternalInput").ap()
    bs_d = nc.dram_tensor("bstack", (5, C), f32, kind="ExternalInput").ap()
    g_d = nc.dram_tensor("Gm", (CC, P, G), f32, kind="ExternalInput").ap()
    gt_d = nc.dram_tensor("GmT", (CC, G, P), f32, kind="ExternalInput").ap()
    out_d = nc.dram_tensor("out", (C, N), f32, kind="ExternalOutput").ap()

    x_r = x_d.rearrange("(cc p) n -> p cc n", p=P)
    out_r = out_d.rearrange("(cc p) n -> p cc n", p=P)

    # index of the one ACT table set covering every func we use
    # (ln, exp, square, copy, identity)
    from concourse.hw_specs import get_activation_tables
    act_sets = list(get_activation_tables(nc.m.arch))
    LNEXP_SET = act_sets.index("natural_log_exp_and_others")

    with tile.TileContext(nc) as tc:
        with (
            tc.tile_pool(name="const", bufs=1) as const,
            tc.tile_pool(name="data", bufs=1) as data,
            tc.tile_pool(name="ps", bufs=1, space="PSUM") as ps,
        ):
            # preload it once at t=0 so no mid-kernel table switches occur
            _ld = mybir.InstLoadActFuncSet(
                name=nc.get_next_instruction_name(), ins=[], outs=[],
                act_func_set_id=LNEXP_SET)
            nc.scalar.add_instruction(_ld)
            # ---- load x in chunks first (DMA-critical), GN stats overlap ----
            NC_ = N // XCH
            xt = data.tile([P, CC, N], f32, tag="x")
            q_t = data.tile([P, CC, N], f32r, tag="q")
            k_t = data.tile([P, CC, N], f32r, tag="k")
            vp_t = data.tile([P, NJT, C], f32r, tag="vp")

            with tc.tile_pool(name="hnp", bufs=1) as hnp:
                # f32r copy of x for the projection matmuls (GroupNorm's
                # affine is folded into the weights/biases, so this needs
                # only the raw x and overlaps the x DMA)
                xr = hnp.tile([P, CC, N], f32r, tag="xr")

                # per-chunk bn_stats records
                st6 = hnp.tile([P, CC, XCH, 6], f32, tag="st6")
                for xc in range(XCH):
                    nsl = slice(xc * NC_, (xc + 1) * NC_)
                    nc.sync.dma_start(xt[:, :, nsl], x_r[:, :, nsl])
                    for cc in range(CC):
                        nc.vector.bn_stats(st6[:, cc, xc], xt[:, cc, nsl])
                        if (xc + cc) % 2 == 0:
                            nc.scalar.activation(xr[:, cc, nsl], xt[:, cc, nsl],
                                                 AF.Copy)
                        else:
                            nc.vector.tensor_copy(xr[:, cc, nsl], xt[:, cc, nsl])

                # ---- constants (after x on the DMA queue) ----
                w_l = hnp.tile([P, 3, CC, C], f32, tag="wl")
                nc.sync.dma_start(w_l[:], ws_d.rearrange("w (cc p) o -> p w cc o", p=P))
                b_t = const.tile([P, 5, CC], f32, tag="bt")
                nc.sync.dma_start(b_t[:], bs_d.rearrange("v (cc p) -> p v cc", p=P))
                g_t = const.tile([P, CC, G], f32, tag="G")
                nc.sync.dma_start(g_t[:], g_d.rearrange("cc p g -> p cc g"))
                gt_t = const.tile([G, CC, P], f32, tag="GT")
                nc.sync.dma_start(gt_t[:], gt_d.rearrange("cc g p -> g cc p"))

                w_r = const.tile([P, 3, CC, C], f32r, tag="wr")
                wq_t, wk_t, wov_t = w_r[:, 0], w_r[:, 1], w_r[:, 2]
                bq_t, bk_t, bo_t, gns_t, gnb_t = (b_t[:, v] for v in range(5))

                ones_l = hnp.tile([P, P], f32, tag="onesl")
                nc.gpsimd.memset(ones_l[:], 1.0)
                ones_t = const.tile([P, P], f32r, tag="ones")
                nc.gpsimd.tensor_copy(ones_t[:], ones_l[:])
                eps_t = const.tile([G, 1], f32, tag="eps")
                nc.gpsimd.memset(eps_t[:], EPS)

                # per-channel (mean, E[x^2]) from aggregated bn records
                mv = const.tile([P, CC, 2], f32, tag="mv")
                stc = const.tile([P, CC, 2], f32, tag="stc")
                for cc in range(CC):
                    nc.vector.bn_aggr(mv[:, cc], st6[:, cc])
                    nc.vector.tensor_copy(stc[:, cc, 0:1], mv[:, cc, 0:1])
                    nc.vector.tensor_mul(stc[:, cc, 1:2], mv[:, cc, 0:1],
                                         mv[:, cc, 0:1])
                    nc.vector.tensor_add(stc[:, cc, 1:2], stc[:, cc, 1:2],
                                         mv[:, cc, 1:2])

                # group-reduce per-channel (mean, E[x^2]) straight in PSUM
                gps = ps.tile([G, 2], f32, tag="ob", bufs=2, name="gps")
                for cc in range(CC):
                    nc.tensor.matmul(gps[:], g_t[:, cc], stc[:, cc],
                                     start=(cc == 0), stop=(cc == CC - 1))

                # grp cols: 0=mean 1=rstd 2=ex2 3=mean^2 4=var 5=sqrt(var+eps)
                CPG = C // G
                grp = const.tile([G, 6], f32, tag="grp")
                nc.vector.tensor_scalar_mul(grp[:, 0:1], gps[:, 0:1], 1.0 / CPG)
                nc.vector.tensor_scalar_mul(grp[:, 2:3], gps[:, 1:2], 1.0 / CPG)
                nc.vector.tensor_mul(grp[:, 3:4], grp[:, 0:1], grp[:, 0:1])
                nc.vector.tensor_sub(grp[:, 4:5], grp[:, 2:3], grp[:, 3:4])
                # rstd = exp(-0.5*ln(var+eps)): keeps every ACT func in the
                # natural_log_exp set -> a single table load for the kernel
                nc.scalar.activation(grp[:, 5:6], grp[:, 4:5], AF.Ln, bias=eps_t[:])
                nc.scalar.activation(grp[:, 1:2], grp[:, 5:6], AF.Exp, scale=-0.5)

                # broadcast group (mean, rstd) to channels; A/B affine coeffs
                ab = const.tile([P, CC, 2], f32, tag="ab")  # 0=A 1=B
                for cc in range(CC):
                    chps = ps.tile([P, 2], f32, tag="ob", bufs=2, name="chps")
                    nc.tensor.matmul(chps[:], gt_t[:, cc], grp[:, 0:2],
                                     start=True, stop=True)
                    # A = rstd * gn_scale
                    nc.vector.tensor_mul(ab[:, cc, 0:1], chps[:, 1:2],
                                         gns_t[:, cc:cc + 1])
                    # B = gn_bias - mean * A
                    nc.vector.tensor_mul(ab[:, cc, 1:2], chps[:, 0:1], ab[:, cc, 0:1])
                    nc.vector.tensor_sub(ab[:, cc, 1:2], gnb_t[:, cc:cc + 1],
                                         ab[:, cc, 1:2])

                # fold GN affine into the weights: w'[ci,:] = w[ci,:]*A[ci]
                # (k first -- the k projection runs first)
                for w in (1, 0, 2):
                    for cc in range(CC):
                        nc.vector.tensor_scalar_mul(w_r[:, w, cc],
                                                    w_l[:, w, cc],
                                                    ab[:, cc, 0:1])

                # fold the w^T @ B terms into the eviction biases (fp32,
                # N=1 matmuls); for wov the term rides the residual bias
                # because softmax rows sum to 1
                bq3 = const.tile([P, CC], f32, tag="bq3")
                bk3 = const.tile([P, CC], f32, tag="bk3")
                bo3 = const.tile([P, CC], f32, tag="bo3")
                for w, (b_in, b_out) in ((1, (bk_t, bk3)), (0, (bq_t, bq3)),
                                         (2, (bo_t, bo3))):
                    for oc in range(CC):
                        bp = ps.tile([P, 1], f32, tag="ob", bufs=2, name="bp")
                        for ci in range(CC):
                            nc.tensor.matmul(bp[:],
                                             w_l[:, w, ci, oc * P:(oc + 1) * P],
                                             ab[:, ci, 1:2],
                                             start=(ci == 0), stop=(ci == CC - 1))
                        nc.vector.tensor_add(b_out[:, oc:oc + 1], bp[:],
                                             b_in[:, oc:oc + 1])

                # xt += bo3 (residual base incl the wov@B passthrough) on
                # GpSimd during the projection phase when Pool is idle
                for cc in range(CC):
                    nc.gpsimd.tensor_scalar_add(xt[:, cc], xt[:, cc],
                                                bo3[:, cc:cc + 1])

                # ---- k projection (evictions on ScalarE) ----
                for oc in range(CC):
                    for ib in range(NIB):
                        pp = ps.tile([P, IB], f32, tag="ob", bufs=2, name="pp")
                        for ci in range(CC):
                            nc.tensor.matmul(
                                pp[:],
                                wk_t[:, ci, oc * P:(oc + 1) * P],
                                xr[:, ci, ib * IB:(ib + 1) * IB],
                                start=(ci == 0), stop=(ci == CC - 1))
                        nc.scalar.activation(k_t[:, oc, ib * IB:(ib + 1) * IB],
                                             pp[:], AF.Identity,
                                             bias=bk3[:, oc:oc + 1])

                # ---- v'T[j, co] = sum_ci hn[ci, j] wovT[ci, co] (no bias) ----
                for jt in range(NJT):
                    pv = ps.tile([P, C], f32, tag="ob", bufs=2, name="pv")
                    for ci in range(CC):
                        nc.tensor.matmul(
                            pv[:],
                            xr[:, ci, jt * P:(jt + 1) * P],
                            wov_t[:, ci, :],
                            start=(ci == 0), stop=(ci == CC - 1))
                    nc.vector.tensor_copy(vp_t[:, jt], pv[:])

                # ---- q projection, i-blocks outermost (attention starts on
                # ib0 as soon as its q slice lands); evictions on VectorE ----
                for ib in range(NIB):
                    for oc in range(CC):
                        pq = ps.tile([P, IB], f32, tag="ob", bufs=2, name="pq")
                        for ci in range(CC):
                            nc.tensor.matmul(
                                pq[:],
                                wq_t[:, ci, oc * P:(oc + 1) * P],
                                xr[:, ci, ib * IB:(ib + 1) * IB],
                                start=(ci == 0), stop=(ci == CC - 1))
                        nc.vector.tensor_scalar_add(
                            q_t[:, oc, ib * IB:(ib + 1) * IB], pq[:],
                            bq3[:, oc:oc + 1])

            with tc.tile_pool(name="work", bufs=1) as work:
                for ib in range(NIB):
                    isl = slice(ib * IB, (ib + 1) * IB)
                    ob = []
                    for co in range(CC):
                        obt = ps.tile([P, IB], f32, tag="ob", bufs=2,
                                      name=f"ob_{ib}_{co}")
                        ob.append(obt)
                    esa = work.tile([P, JG, IB], f32r, tag="esum", bufs=4, name="esa")
                    esb = work.tile([P, JG, IB], f32r, tag="esum", bufs=4, name="esb")

                    def emit_scores(g):
                        ssg = ps.tile([P, JG, IB], f32, tag="score", bufs=3,
                                      name="ssg")
                        for t in range(JG):
                            jt = g * JG + t
                            for ci in range(CC):
                                nc.tensor.matmul(
                                    ssg[:, t],
                                    k_t[:, ci, jt * P:(jt + 1) * P],
                                    q_t[:, ci, isl],
                                    start=(ci == 0), stop=(ci == CC - 1))
                        return ssg

                    # emit two score groups ahead so the PE stream never has
                    # a pv queued head-of-line behind an unfinished exp
                    ssgs = {0: emit_scores(0), 1: emit_scores(1)}
                    for g in range(NGRP):
                        ssg = ssgs.pop(g)
                        et = work.tile([P, JG, IB], f32r, tag="exp", bufs=4, name="et")
                        if ib == NIB - 1 and g == NGRP - 1:
                            for t in range(JG):
                                nc.scalar.activation(et[:, t], ssg[:, t], AF.Exp)
                        else:
                            nc.scalar.activation(et[:], ssg[:], AF.Exp)
                        if g + 2 < NGRP:
                            ssgs[g + 2] = emit_scores(g + 2)
                        for t in range(JG):
                            jt = g * JG + t
                            for co in range(CC):
                                nc.tensor.matmul(
                                    ob[co][:],
                                    vp_t[:, jt, co * P:(co + 1) * P],
                                    et[:, t],
                                    start=(jt == 0), stop=(jt == NJT - 1))
                        last_direct = (ib == NIB - 1 and g == NGRP - 1)
                        if last_direct:
                            et_last = et
                        elif g == 0:
                            nc.gpsimd.tensor_copy(esb[:], et[:].bitcast(f32))
                        elif g < NGRP // 2:
                            nc.gpsimd.tensor_add(esb[:], esb[:].bitcast(f32),
                                                 et[:].bitcast(f32))
                        elif g == NGRP // 2:
                            nc.vector.tensor_copy(esa[:], et[:].bitcast(f32))
                        else:
                            nc.vector.tensor_add(esa[:], esa[:].bitcast(f32),
                                                 et[:].bitcast(f32))

                    # free the ob psum slots right away via ScalarE copies
                    obs = []
                    for co in range(CC):
                        ot = work.tile([P, IB], f32, tag="obs", bufs=4,
                                       name=f"obs_{ib}_{co}")
                        nc.scalar.activation(ot[:], ob[co][:], AF.Copy)
                        obs.append(ot)

                    # denominators: partials reduced over partitions (and
                    # broadcast to 128 partitions) by accumulating matmuls
                    smt = ps.tile([P, JG, IB], f32, tag="score", bufs=3, name="smt")
                    sm = smt[:, 0]
                    parts = [esb, esa] + ([et_last] if ib == NIB - 1 else [])
                    for z, es in enumerate(parts):
                        nc.tensor.matmul(sm, ones_t[:], es[:, 0],
                                         start=(z == 0), stop=False)
                        nc.tensor.matmul(sm, ones_t[:], es[:, 1],
                                         start=False, stop=(z == len(parts) - 1))
                    rec = work.tile([P, IB], f32, tag="rec", bufs=2, name="rec")
                    nc.vector.reciprocal(rec[:], sm)

                    for co in range(CC):
                        on_t = work.tile([P, IB], f32, tag="on", bufs=3, name="on_t")
                        nc.vector.tensor_mul(on_t[:], obs[co][:], rec[:])
                        fin = work.tile([P, IB], f32, tag="fin", bufs=3, name="fin")
                        if ib == NIB - 1 and co == 0:
                            nc.gpsimd.tensor_add(fin[:], on_t[:], xt[:, co, isl])
                        else:
                            nc.vector.tensor_add(fin[:], on_t[:], xt[:, co, isl])
                        if ib == NIB - 1 and co == 1:
                            nc.scalar.dma_start(out_r[:, co, isl], fin[:])
                        else:
                            nc.sync.dma_start(out_r[:, co, isl], fin[:])

    nc.compile()
    nc.m = get_hw_module(nc.m)
    return nc


def _get_nc():
    if "nc" not in _CACHE:
        _CACHE["nc"] = _build()
    return _CACHE["nc"]


def _prep_inputs(x, gn_scale, gn_bias, wq, bq, wk, bk, wv, bv, wo, bo):
    f = np.float32
    x = np.asarray(x, f)
    b = x.shape[0]
    scale = 1.0 / np.sqrt(np.float64(C))
    wqT = (np.asarray(wq, np.float64) * scale).T
    bq2 = (np.asarray(bq, np.float64) * scale).astype(f)
    wkT = np.asarray(wk, np.float64).T
    wovT = (np.asarray(wo, np.float64) @ np.asarray(wv, np.float64)).T
    bo2 = (np.asarray(bo, np.float64)
           + np.asarray(wo, np.float64) @ np.asarray(bv, np.float64)).astype(f)
    wstack = np.ascontiguousarray(
        np.stack([wqT, wkT, wovT]).astype(f))
    bstack = np.ascontiguousarray(np.stack(
        [bq2, np.asarray(bk, f), bo2, np.asarray(gn_scale, f),
         np.asarray(gn_bias, f)]))

    gm = np.zeros((CC, P, G), f)
    for cc in range(CC):
        for p in range(P):
            gm[cc, p, (cc * P + p) // (C // G)] = 1.0
    gmT = np.ascontiguousarray(np.transpose(gm, (0, 2, 1)))

    shared = {"wstack": wstack, "bstack": bstack, "Gm": gm, "GmT": gmT}
    in_maps = []
    for i in range(b):
        m = dict(shared)
        m["x"] = np.ascontiguousarray(x[i].reshape(C, N))
        in_maps.append(m)
    return in_maps


def _run(in_maps, trace=False, trace_cores=None):
    from concourse import bass_utils
    nc = _get_nc()
    return bass_utils.run_bass_kernel_spmd(
        nc, in_maps, core_ids=list(range(len(in_maps))),
        trace=trace, trace_cores=trace_cores)


def kernel(x, gn_scale, gn_bias, wq, bq, wk, bk, wv, bv, wo, bo):
    in_maps = _prep_inputs(x, gn_scale, gn_bias, wq, bq, wk, bk, wv, bv, wo, bo)
    res = _run(in_maps)
    b = np.asarray(x).shape[0]
    out = np.stack([res.results[i]["out"].reshape(C, H, W) for i in range(b)])
    return out.astype(np.float32)
